# revision 1
# baseline (speedup 1.0000x reference)
"""Trainium2 Bass kernel for nn_AttentionBlock_33724083208839 (sparse_attention).

Data-parallel over batch (8 batches -> 8 cores). Per core:
  1. load x (chunked), transpose via PE -> xT f32; project Q^T/K^T/V in f32
     on PE (bf16 copies for the attention matmuls, f32 K kept for selection).
  2. K_reduce via the exact CVaR identity sum_top_l = l*t + sum(relu(x-t)),
     with t from Gaussian quantile + one Newton step on the exact count
     (all on DVE, overlapped with the attention stream).
  3. sqk = x @ (Wq @ K_reduce) in f32 on PE; query top-l as a mask via a
     5-pass 128-ary threshold search on a partition-replicated copy of sqk
     (one fused DVE compare+count per pass; cross-partition reduction on
     GPSIMD so the PE/ACT pipelines never block).
  4. attention for all 4096 queries: scores^T on PE (bf16) -> exp on ACT
     (1536-wide PSUM strips -> SBUF bf16) -> [V|1]^T @ P^T accumulation on
     PE -> transpose back, normalize per row, blend with meanV by the mask.
"""
import sys

sys.path.insert(0, "/opt/trn_rl_repo")

import math
from statistics import NormalDist

import numpy as np

import concourse.bacc as bacc
import concourse.bass as bass
import concourse.bass_isa as bass_isa
import concourse.mybir as mybir
from concourse.tile import TileContext
from concourse.masks import make_identity
from concourse.bass_utils import run_bass_kernel_spmd

B, L, D = 8, 4096, 64
LQ = int((1.0 - 0.33) * L)  # 2744
PART = 128
NT = L // PART
NS = L // 512
N_CORES = 8

QFRAC = 1.0 - LQ / L
Z = NormalDist().inv_cdf(QFRAC)
PHI = math.exp(-Z * Z / 2.0) / math.sqrt(2.0 * math.pi)

f32 = mybir.dt.float32
bf16 = mybir.dt.bfloat16
u8 = mybir.dt.uint8
i32 = mybir.dt.int32
AF = mybir.ActivationFunctionType
OP = mybir.AluOpType

N_PASS = 5
BOUND = 512.0

GROUPS = [(g, min(3, NT - g)) for g in range(0, NT, 3)]


def build(debug: bool = False):
    nc = bacc.Bacc("TRN2")
    x = nc.dram_tensor("x", [L, D], f32, kind="ExternalInput")
    wq = nc.dram_tensor("Wq", [D, D], f32, kind="ExternalInput")
    wk = nc.dram_tensor("Wk", [D, D], f32, kind="ExternalInput")
    wv = nc.dram_tensor("Wv", [D, D], f32, kind="ExternalInput")
    out = nc.dram_tensor("out", [L, D], f32, kind="ExternalOutput")
    scr_row = nc.dram_tensor("scr_row", [1, L], f32, kind="Internal")
    dbg = {}
    if debug:
        for name, shape in [
            ("dbg_kr", [D, 1]), ("dbg_thr", [PART, 1]), ("dbg_sqk", [PART, NT]),
            ("dbg_mask", [PART, NT]), ("dbg_tk", [PART, 1]), ("dbg_cnt", [PART, 1]),
        ]:
            dbg[name] = nc.dram_tensor(name, shape, f32, kind="ExternalOutput")

    x_re = x[:].rearrange("(c p) d -> p c d", p=PART)
    out_re = out[:].rearrange("(c p) d -> p c d", p=PART)

    with TileContext(nc) as tc, \
         tc.tile_pool(name="cst", bufs=1) as cst, \
         tc.tile_pool(name="big", bufs=1) as big, \
         tc.tile_pool(name="sc", bufs=2) as sc, \
         tc.tile_pool(name="mn", bufs=2) as mn:

        # ---- warm the exp activation table immediately ----
        warm = cst.tile([1, 8], f32)
        nc.vector.memset(warm[:], 0.0)
        warm2 = cst.tile([1, 8], f32)
        nc.scalar.activation(out=warm2[:], in_=warm[:], func=AF.Exp)

        # ---- constants ----
        ident = cst.tile([PART, PART], f32)
        make_identity(nc, ident[:])
        onesb = cst.tile([PART, 1], bf16)
        nc.vector.memset(onesb[:], 1.0)
        ones1x128 = cst.tile([1, PART], f32)
        nc.vector.memset(ones1x128[:], 1.0)
        pidx1i = cst.tile([PART, 1], i32)
        nc.gpsimd.iota(pidx1i[:], pattern=[[1, 1]], base=1, channel_multiplier=1)
        pidx1 = cst.tile([PART, 1], f32)
        nc.vector.tensor_copy(pidx1[:], pidx1i[:])

        # ---- persistent tensors ----
        x_sb = big.tile([PART, NT, D], f32)
        xT32 = big.tile([D, L], f32)
        qT = big.tile([D, L], bf16)
        kT = big.tile([D, L], bf16)
        kT32 = big.tile([D, L], f32)
        vp = big.tile([PART, NT, D + 1], bf16)
        pt_a = big.tile([PART, NT, 512], bf16)
        pt_b = big.tile([PART, NT, 512], bf16)
        res = big.tile([PART, NT, D], f32)
        mvf = big.tile([PART, D], f32)
        mask = big.tile([PART, NT], f32)
        mask_u8 = big.tile([PART, NT], u8)
        sqk = big.tile([PART, NT], f32)
        kr = big.tile([D, 1], f32)
        wvec = big.tile([D, 1], f32)
        tmp1m = big.tile([D, L], f32)
        sqk_rep = big.tile([PART, L], f32)
        cmp_rep = big.tile([PART, L], f32)

        # weights
        wq_s = cst.tile([D, D], f32)
        wk_s = cst.tile([D, D], f32)
        wv_s = cst.tile([D, D], f32)
        nc.sync.dma_start(out=wq_s[:], in_=wq[:])
        nc.sync.dma_start(out=wk_s[:], in_=wk[:])
        nc.sync.dma_start(out=wv_s[:], in_=wv[:])

        # =============== phase 1: load / project / slab-0 scores+exp ===============
        with tc.tile_pool(name="ps_xv", bufs=2, space="PSUM") as ps_xv, \
             tc.tile_pool(name="ps_pj", bufs=2, space="PSUM") as ps_pj, \
             tc.tile_pool(name="ps_s0", bufs=1, space="PSUM") as ps_s0:

            def load_tiles(c0, c1):
                nc.sync.dma_start(out=x_sb[:, c0:c1, :], in_=x_re[:, c0:c1, :])
                for c in range(c0, c1):
                    pxt = ps_xv.tile([PART, PART], f32, tag="xv")
                    nc.tensor.transpose(out=pxt[0:D, :], in_=x_sb[:, c, :],
                                        identity=ident[:])
                    nc.vector.tensor_copy(xT32[:, PART * c:PART * (c + 1)], pxt[0:D, :])

            def proj_slab(s):
                sl = slice(512 * s, 512 * (s + 1))
                pk = ps_pj.tile([D, 512], f32, tag="pj")
                nc.tensor.matmul(out=pk[:], lhsT=wk_s[:], rhs=xT32[:, sl],
                                 start=True, stop=True)
                nc.vector.tensor_copy(kT32[:, sl], pk[:])
                nc.scalar.copy(kT[:, sl], pk[:])
                pq = ps_pj.tile([D, 512], f32, tag="pj")
                nc.tensor.matmul(out=pq[:], lhsT=wq_s[:], rhs=xT32[:, sl],
                                 start=True, stop=True)
                nc.scalar.copy(qT[:, sl], pq[:])

            def sg0(gi, grp=None):
                g0, glen = grp if grp is not None else GROUPS[gi]
                strip = ps_s0.tile([PART, 3, 512], f32, tag="s0")
                for i in range(glen):
                    j = g0 + i
                    nc.tensor.matmul(out=strip[:, i, :],
                                     lhsT=kT[:, PART * j:PART * (j + 1)],
                                     rhs=qT[:, 0:512], start=True, stop=True)
                nc.scalar.activation(out=pt_a[:, g0:g0 + glen, :],
                                     in_=strip[:, 0:glen, :], func=AF.Exp, scale=0.125)

            def proj_v(c0, c1):
                for c in range(c0, c1):
                    pv = ps_xv.tile([PART, PART], f32, tag="xv")
                    nc.tensor.matmul(out=pv[:, 0:D],
                                     lhsT=xT32[:, PART * c:PART * (c + 1)],
                                     rhs=wv_s[:], start=True, stop=True)
                    nc.vector.tensor_copy(vp[:, c, 0:D], pv[:, 0:D])

            nc.sync.dma_start(out=x_sb[:, 0:2, :], in_=x_re[:, 0:2, :])
            for c in (0, 1):
                pxt = ps_xv.tile([PART, PART], f32, tag="xv")
                nc.tensor.transpose(out=pxt[0:D, :], in_=x_sb[:, c, :],
                                    identity=ident[:])
                nc.vector.tensor_copy(xT32[:, PART * c:PART * (c + 1)], pxt[0:D, :])
            pk0 = ps_pj.tile([D, 512], f32, tag="pj")
            nc.tensor.matmul(out=pk0[:, 0:PART], lhsT=wk_s[:], rhs=xT32[:, 0:PART],
                             start=True, stop=True)
            nc.vector.tensor_copy(kT32[:, 0:PART], pk0[:, 0:PART])
            nc.scalar.copy(kT[:, 0:PART], pk0[:, 0:PART])
            pq0 = ps_pj.tile([D, 512], f32, tag="pj")
            nc.tensor.matmul(out=pq0[:, 0:PART * 2], lhsT=wq_s[:],
                             rhs=xT32[:, 0:PART * 2], start=True, stop=True)
            nc.scalar.copy(qT[:, 0:PART * 2], pq0[:, 0:PART * 2])
            load_tiles(2, 4)
            # queries 0-255 of slab 0 against k-tile 0 can fire immediately;
            # the remaining kT/qT columns of slab 0 follow right after
            pk1 = ps_pj.tile([D, 512], f32, tag="pj")
            nc.tensor.matmul(out=pk1[:, 0:384], lhsT=wk_s[:], rhs=xT32[:, PART:512],
                             start=True, stop=True)
            nc.vector.tensor_copy(kT32[:, PART:512], pk1[:, 0:384])
            nc.scalar.copy(kT[:, PART:512], pk1[:, 0:384])
            pq1 = ps_pj.tile([D, 512], f32, tag="pj")
            nc.tensor.matmul(out=pq1[:, 0:256], lhsT=wq_s[:], rhs=xT32[:, 256:512],
                             start=True, stop=True)
            nc.scalar.copy(qT[:, 256:512], pq1[:, 0:256])
            proj_v(0, 4)
            sg0(0, grp=(0, 1))
            sg0(0, grp=(1, 2))
            load_tiles(4, 8)
            proj_slab(1)
            proj_v(4, 8)
            sg0(1)
            load_tiles(8, 12)
            load_tiles(12, 16)
            proj_slab(2); proj_v(8, 12); sg0(2)
            proj_slab(3); proj_v(12, 16); sg0(3); sg0(4)
            load_tiles(16, 20)
            load_tiles(20, 24)
            proj_slab(4); proj_v(16, 20); sg0(5)
            proj_slab(5); proj_v(20, 24); sg0(6); sg0(7)
            load_tiles(24, 28)
            load_tiles(28, 32)
            proj_slab(6); proj_v(24, 28); sg0(8)
            proj_slab(7); proj_v(28, 32)
            nc.vector.memset(vp[:, :, D:D + 1], 1.0)
            sg0(9)
            sg0(10)

        # ---- K_reduce (DVE only; channel = partition of kT32) ----
        bstats = sc.tile([D, 8, 6], f32, tag="bstats")
        for a in range(8):
            nc.vector.bn_stats(bstats[:, a, :], kT32[:, 512 * a:512 * (a + 1)])
        aggr = sc.tile([D, 2], f32, tag="aggr")
        nc.vector.bn_aggr(aggr[:], bstats[:])
        sig = sc.tile([D, 1], f32, tag="sig")
        nc.vector.memset(sig[:], 1.0)
        for _ in range(4):
            rs = sc.tile([D, 1], f32, tag="rs")
            nc.vector.reciprocal(rs[:], sig[:])
            nc.vector.tensor_tensor(out=rs[:], in0=rs[:], in1=aggr[:, 1:2], op=OP.mult)
            nc.vector.tensor_tensor(out=rs[:], in0=rs[:], in1=sig[:], op=OP.add)
            nc.vector.tensor_scalar_mul(sig[:], rs[:], 0.5)
        tk = sc.tile([D, 1], f32, tag="tk")
        nc.vector.tensor_scalar(out=tk[:], in0=sig[:], scalar1=float(Z),
                                scalar2=None, op0=OP.mult)
        nc.vector.tensor_tensor(out=tk[:], in0=tk[:], in1=aggr[:, 0:1], op=OP.add)
        cnt_c = sc.tile([D, 1], f32, tag="cnt_c")
        nc.vector.tensor_scalar(out=tmp1m[:], in0=kT32[:], scalar1=tk[:, 0:1],
                                scalar2=None, op0=OP.is_gt, op1=OP.add,
                                accum_out=cnt_c[:])
        adj = sc.tile([D, 1], f32, tag="adj")
        nc.vector.tensor_scalar(out=adj[:], in0=cnt_c[:], scalar1=float(-LQ),
                                scalar2=1.0 / (L * PHI), op0=OP.add, op1=OP.mult)
        nc.vector.tensor_tensor(out=adj[:], in0=adj[:], in1=sig[:], op=OP.mult)
        t1 = sc.tile([D, 1], f32, tag="t1")
        nc.vector.tensor_tensor(out=t1[:], in0=tk[:], in1=adj[:], op=OP.add)
        nc.vector.tensor_scalar(out=tmp1m[:], in0=kT32[:], scalar1=t1[:, 0:1],
                                scalar2=0.0, op0=OP.subtract, op1=OP.max)
        s1c = sc.tile([D, 1], f32, tag="s1c")
        nc.vector.tensor_reduce(out=s1c[:], in_=tmp1m[:], axis=mybir.AxisListType.X,
                                op=OP.add)
        nc.vector.tensor_scalar(out=kr[:], in0=s1c[:], scalar1=1.0 / LQ,
                                scalar2=None, op0=OP.mult)
        nc.vector.tensor_tensor(out=kr[:], in0=kr[:], in1=t1[:], op=OP.add)

        # =============== phase 2: attention + selection ===============
        with tc.tile_pool(name="ps_strip", bufs=2, space="PSUM") as ps_strip, \
             tc.tile_pool(name="ps_acc", bufs=1, space="PSUM") as ps_acc, \
             tc.tile_pool(name="ps_mis", bufs=1, space="PSUM") as ps_mis:

            def pt_of(s):
                return pt_a if s % 2 == 0 else pt_b

            def emit_slab_lag(s, extra=None):
                """scores+exp for slab s (if s < NS) interleaved with AV of
                slab s-1; returns oT_sb of slab s-1."""
                oT = ps_acc.tile([D + 1, 512], f32, tag="oT")
                ptp = pt_of(s - 1)
                ptc = pt_of(s)
                for (g0, glen) in GROUPS:
                    if extra is not None:
                        extra(g0, glen)
                    if s < NS:
                        strip = ps_strip.tile([PART, 3, 512], f32, tag="strip")
                        for i in range(glen):
                            j = g0 + i
                            nc.tensor.matmul(out=strip[:, i, :],
                                             lhsT=kT[:, PART * j:PART * (j + 1)],
                                             rhs=qT[:, 512 * s:512 * (s + 1)],
                                             start=True, stop=True)
                        nc.scalar.activation(out=ptc[:, g0:g0 + glen, :],
                                             in_=strip[:, 0:glen, :], func=AF.Exp,
                                             scale=0.125)
                    for i in range(glen):
                        j = g0 + i
                        nc.tensor.matmul(out=oT[:], lhsT=vp[:, j, :], rhs=ptp[:, j, :],
                                         start=(j == 0), stop=(j == NT - 1))
                oT_sb = mn.tile([D + 1, 512], f32, tag="oT_sb")
                nc.vector.tensor_copy(oT_sb[:], oT[:])
                return oT_sb

            def emit_blend(s, oT_sb):
                for i in range(4):
                    c = 4 * s + i
                    po = ps_mis.tile([PART, 512], f32, tag="mis")
                    nc.tensor.transpose(out=po[:, 0:D + 1],
                                        in_=oT_sb[:, PART * i:PART * (i + 1)],
                                        identity=ident[0:D + 1, 0:D + 1])
                    dcol = mn.tile([PART, 1], f32, tag="dcol")
                    nc.vector.tensor_copy(dcol[:], po[:, D:D + 1])
                    rec = mn.tile([PART, 1], f32, tag="rec")
                    nc.vector.reciprocal_approx_fast(rec[:], dcol[:])
                    tnorm = mn.tile([PART, D], f32, tag="tnorm")
                    nc.vector.tensor_scalar(out=tnorm[:], in0=po[:, 0:D],
                                            scalar1=rec[:, 0:1], scalar2=None,
                                            op0=OP.mult)
                    nc.vector.tensor_copy(res[:, c, :], mvf[:])
                    nc.vector.copy_predicated(res[:, c, :],
                                              mask_u8[:, c:c + 1].to_broadcast([PART, D]),
                                              tnorm[:])
                    nc.sync.dma_start(out=out_re[:, c:c + 1, :], in_=res[:, c:c + 1, :])

            # ---- slab 1 (+ AV of slab 0) ----
            oT_sbs = {}
            oT_sbs[0] = emit_slab_lag(1)

            # meanV: accumulate on PE (mis bank), then finalize
            pmv = ps_mis.tile([PART, 512], f32, tag="mis")
            for c in range(NT):
                nc.tensor.matmul(out=pmv[0:D + 1, 0:1], lhsT=vp[:, c, :], rhs=onesb[:],
                                 start=(c == 0), stop=(c == NT - 1))
            mv_col = sc.tile([D, 1], f32, tag="mv_col")
            nc.vector.tensor_scalar_mul(mv_col[:], pmv[0:D, 0:1], 1.0 / L)
            pmvT = ps_mis.tile([PART, 512], f32, tag="mis")
            nc.tensor.transpose(out=pmvT[0:1, 0:D], in_=mv_col[:],
                                identity=ident[0:D, 0:D])
            mv_row = sc.tile([1, D], f32, tag="mv_row")
            nc.vector.tensor_copy(mv_row[:], pmvT[0:1, 0:D])
            pmvF = ps_mis.tile([PART, 512], f32, tag="mis")
            nc.tensor.matmul(out=pmvF[:, 0:D], lhsT=ones1x128[:], rhs=mv_row[:],
                             start=True, stop=True)
            nc.vector.tensor_copy(mvf[:], pmvF[:, 0:D])

            def emit_selection():
                # ---- w = Wq @ Kr ; sqk ----
                pwt = ps_mis.tile([PART, 512], f32, tag="mis")
                nc.tensor.transpose(out=pwt[0:D, 0:D], in_=wq_s[:], identity=ident[0:D, 0:D])
                wqT = sc.tile([D, D], f32, tag="wqT")
                nc.vector.tensor_copy(wqT[:], pwt[0:D, 0:D])
                pw = ps_mis.tile([PART, 512], f32, tag="mis")
                nc.tensor.matmul(out=pw[0:D, 0:1], lhsT=wqT[:], rhs=kr[:],
                                 start=True, stop=True)
                nc.vector.tensor_copy(wvec[:], pw[0:D, 0:1])
                psq = ps_mis.tile([PART, 512], f32, tag="mis")
                for c in range(NT):
                    nc.tensor.matmul(out=psq[:, c:c + 1],
                                     lhsT=xT32[:, PART * c:PART * (c + 1)],
                                     rhs=wvec[:], start=True, stop=True)
                nc.vector.tensor_copy(sqk[:], psq[:, 0:NT])

                # replicate sqk into every partition via DRAM round-trip
                psqT = ps_mis.tile([PART, 512], f32, tag="mis")
                nc.tensor.transpose(out=psqT[0:NT, 0:PART], in_=sqk[:], identity=ident[:])
                sqkT = sc.tile([NT, PART], f32, tag="sqkT")
                nc.vector.tensor_copy(sqkT[:], psqT[0:NT, 0:PART])
                nc.sync.dma_start(out=scr_row[:], in_=sqkT[:])
                nc.sync.dma_start(out=sqk_rep[:], in_=scr_row[:].to_broadcast([PART, L]))

                if debug:
                    nc.sync.dma_start(out=dbg["dbg_kr"][:], in_=kr[:])
                    nc.sync.dma_start(out=dbg["dbg_sqk"][:], in_=sqk[:])
                    nc.sync.dma_start(out=dbg["dbg_tk"][0:D, :], in_=t1[:])

                # ---- 5-pass 128-ary threshold search ----
                lo = mn.tile([PART, 1], f32, tag="lo_a")
                nc.vector.memset(lo[:], -BOUND)
                dlt = mn.tile([PART, 1], f32, tag="dlt_a")
                nc.vector.memset(dlt[:], 2.0 * BOUND / 129.0)
                for it in range(N_PASS):
                    tvec = mn.tile([PART, 1], f32, tag=f"tv{it % 2}")
                    nc.vector.tensor_tensor(out=tvec[:], in0=pidx1[:], in1=dlt[:], op=OP.mult)
                    nc.vector.tensor_tensor(out=tvec[:], in0=tvec[:], in1=lo[:], op=OP.add)
                    cntq = mn.tile([PART, 1], f32, tag="cntq")
                    nc.vector.tensor_scalar(out=cmp_rep[:], in0=sqk_rep[:],
                                            scalar1=tvec[:, 0:1], scalar2=None,
                                            op0=OP.is_gt, op1=OP.add, accum_out=cntq[:])
                    sel = mn.tile([PART, 1], f32, tag="sel")
                    nc.vector.tensor_scalar(out=sel[:], in0=cntq[:], scalar1=float(LQ),
                                            scalar2=None, op0=OP.is_ge)
                    jsr = mn.tile([PART, 1], f32, tag="jsr")
                    nc.gpsimd.partition_all_reduce(jsr[:], sel[:], channels=PART,
                                                   reduce_op=bass_isa.ReduceOp.add)
                    step = mn.tile([PART, 1], f32, tag="step")
                    nc.vector.tensor_tensor(out=step[:], in0=jsr[:], in1=dlt[:], op=OP.mult)
                    nlo = mn.tile([PART, 1], f32, tag=f"lo_{'b' if it % 2 == 0 else 'a'}")
                    nc.vector.tensor_tensor(out=nlo[:], in0=lo[:], in1=step[:], op=OP.add)
                    ndl = mn.tile([PART, 1], f32, tag=f"dlt_{'b' if it % 2 == 0 else 'a'}")
                    nc.vector.tensor_scalar_mul(ndl[:], dlt[:], 1.0 / 129.0)
                    lo, dlt = nlo, ndl
                nc.vector.tensor_scalar(out=mask[:], in0=sqk[:], scalar1=lo[:, 0:1],
                                        scalar2=None, op0=OP.is_gt)
                nc.vector.tensor_copy(mask_u8[:], mask[:])
                if debug:
                    nc.sync.dma_start(out=dbg["dbg_mask"][:], in_=mask[:])
                    nc.sync.dma_start(out=dbg["dbg_thr"][:], in_=lo[:])
                    cntf = mn.tile([PART, 1], f32, tag="cntf")
                    cmpf = mn.tile([PART, NT], f32, tag="cmpf")
                    nc.vector.tensor_scalar(out=cmpf[:], in0=sqk[:], scalar1=lo[:, 0:1],
                                            scalar2=None, op0=OP.is_gt, op1=OP.add,
                                            accum_out=cntf[:])
                    nc.sync.dma_start(out=dbg["dbg_cnt"][:], in_=cntf[:])


            # ---- slabs 2..7 + final AV-only pass; blends lag two slabs ----
            emit_selection()
            for s in range(2, NS + 1):
                oT_sbs[s - 1] = emit_slab_lag(s)
                if s - 2 in oT_sbs:
                    emit_blend(s - 2, oT_sbs.pop(s - 2))
            for s_ in sorted(oT_sbs):
                emit_blend(s_, oT_sbs.pop(s_))

    nc.finalize()
    return nc


_CACHE = {}


def _get_nc(debug=False):
    key = bool(debug)
    if key not in _CACHE:
        _CACHE[key] = build(debug=key)
    return _CACHE[key]


def kernel(x, Wq, Wk, Wv, debug=False):
    nc = _get_nc(debug=debug)
    x = np.asarray(x, dtype=np.float32)
    in_maps = [
        {"x": np.ascontiguousarray(x[i]),
         "Wq": np.asarray(Wq, np.float32), "Wk": np.asarray(Wk, np.float32),
         "Wv": np.asarray(Wv, np.float32)}
        for i in range(B)
    ]
    last_err = None
    for _attempt in range(3):
        try:
            r = run_bass_kernel_spmd(nc, in_maps, core_ids=list(range(N_CORES)))
            out = np.stack([r.results[i]["out"] for i in range(B)]).astype(np.float32)
            break
        except Exception as e:  # transient axon RPC failures
            last_err = e
    else:
        raise last_err
    if debug:
        return out, r.results
    return out

